# revision 18
# baseline (speedup 1.0000x reference)
import numpy as np

# nn_LocalDynamics GNN message passing.
# delta[n] = sum_e tanh(fMLP(inp_e))[addr_from=n] + tanh(tMLP(inp_e))[addr_to=n]
# out = tanh(delta).  inp_e = [h[from], h[to], x_e, hg, xg, t] (153 dims).
#
# Device: per-core fp16 MLP over EPC=100k edges (feature-major [128, E] tiles),
# fp16 matmuls (1 cyc/row) + fp32 PSUM + tanh on ACT.  Host: gather/scatter.
N = 100_000
E = 800_000
D = 64
H = 128
NCORES = 8
PAIR = 1024                      # edges per device iteration
EPC = E // NCORES                # 100000 edges per core
EPAD = ((EPC + PAIR - 1) // PAIR) * PAIR   # 100352


def _scatter_add(delta, idx, vals):
    o = np.argsort(idx, kind="stable")
    si = idx[o]
    sv = vals[o]
    starts = np.flatnonzero(np.r_[True, si[1:] != si[:-1]])
    sums = np.add.reduceat(sv, starts, axis=0)
    np.add.at(delta, si[starts], sums)


def _host_post(af, at, d_from, d_to, n_nodes):
    delta = np.zeros((n_nodes, D), dtype=np.float32)
    _scatter_add(delta, af, d_from)
    _scatter_add(delta, at, d_to)
    return np.tanh(delta).astype(np.float32)


def _mlp_np(x, W0, b0, W1, b1, W2, b2):
    h = np.tanh(x @ W0 + b0)
    h = np.tanh(h @ W1 + b1)
    return h @ W2 + b2


def _kernel_numpy(addr_from, addr_to, h_local, h_global, x_local, x_global, t,
                  f_W0, f_b0, f_W1, f_b1, f_W2, f_b2,
                  t_W0, t_b0, t_W1, t_b1, t_W2, t_b2):
    af = np.asarray(addr_from).astype(np.int64)
    at = np.asarray(addr_to).astype(np.int64)
    h_local = np.asarray(h_local, dtype=np.float32)
    x_local = np.asarray(x_local, dtype=np.float32)
    const = np.concatenate([np.asarray(h_global, np.float32).ravel(),
                            np.asarray(x_global, np.float32).ravel(),
                            np.asarray(t, np.float32).ravel()])
    ne = af.shape[0]
    d_from = np.empty((ne, D), np.float32)
    d_to = np.empty((ne, D), np.float32)
    CH = 100_000
    for s in range(0, ne, CH):
        e = min(s + CH, ne)
        inp = np.concatenate([h_local[af[s:e]], h_local[at[s:e]], x_local[s:e],
                              np.broadcast_to(const, (e - s, 21))], axis=1).astype(np.float32)
        d_from[s:e] = np.tanh(_mlp_np(inp, f_W0, f_b0, f_W1, f_b1, f_W2, f_b2))
        d_to[s:e] = np.tanh(_mlp_np(inp, t_W0, t_b0, t_W1, t_b1, t_W2, t_b2))
    return _host_post(af, at, d_from, d_to, h_local.shape[0])


_BASS_CACHE = {}


def _build_bass():
    import concourse.bass as bass
    import concourse.mybir as mybir
    import concourse.tile as tile

    # walrus in this env rejects Drain instructions carrying >1 sem wait;
    # move each wait onto its own sync nop before the drain.
    def _patched(self, tick_clock, wait_clock):
        from concourse.tile import ScopedClock
        nop0 = self.nc.sync.nop(nofuse=True)
        wait_clock.add_sem_waits(nop0.ins, ScopedClock({None: tick_clock.global_clock}))
        si = nop0.ins.sync_info
        if si is not None and si.on_wait and len(si.on_wait) > 1:
            waits = list(si.on_wait)
            si.on_wait = waits[:1]
            for w in waits[1:]:
                n = self.nc.sync.nop(nofuse=True)
                n.ins.sync_info = mybir.SyncInfo(on_wait=[w], on_update=[])
        self.nc.sync.drain()
        self.nc.all_engine_barrier()
        popped = self.nc._tile_sem_poison_stack.pop()
        assert popped is self._sem_poison
        self.nc.clear_and_free_semaphores(list(self.sems.allocated().values()))
        self.nc.all_engine_barrier()

    tile.TileContext._drain_and_barrier = _patched

    f16 = mybir.dt.float16
    f32 = mybir.dt.float32
    nc = bass.Bass()
    inpa = nc.dram_tensor("inpa", [128, EPAD], f16, kind="ExternalInput")
    inpb = nc.dram_tensor("inpb", [4, EPAD], f16, kind="ExternalInput")
    wts = {}
    for p in ("f", "t"):
        wts[p + "w0a"] = nc.dram_tensor(p + "w0a", [128, H], f16, kind="ExternalInput")
        wts[p + "w0b"] = nc.dram_tensor(p + "w0b", [4, H], f16, kind="ExternalInput")
        wts[p + "w1"] = nc.dram_tensor(p + "w1", [H, H], f16, kind="ExternalInput")
        wts[p + "w2"] = nc.dram_tensor(p + "w2", [H, D], f16, kind="ExternalInput")
        wts[p + "b0"] = nc.dram_tensor(p + "b0", [H, 1], f32, kind="ExternalInput")
        wts[p + "b1"] = nc.dram_tensor(p + "b1", [H, 1], f32, kind="ExternalInput")
    wts["bcat"] = nc.dram_tensor("bcat", [128, 1], f32, kind="ExternalInput")
    dout = nc.dram_tensor("d", [128, EPAD], f16, kind="ExternalOutput")

    Tanh = mybir.ActivationFunctionType.Tanh
    with tile.TileContext(nc) as tc:
        with tc.tile_pool(name="wpool", bufs=1) as wp, \
             tc.tile_pool(name="io", bufs=4) as io, \
             tc.tile_pool(name="iob", bufs=4) as iob, \
             tc.tile_pool(name="act", bufs=4) as ap_, \
             tc.tile_pool(name="dtp", bufs=3) as dtp, \
             tc.tile_pool(name="ps1", bufs=2, space="PSUM") as ps1p, \
             tc.tile_pool(name="ps2", bufs=2, space="PSUM") as ps2p:
            wt = {}
            for k, dr in wts.items():
                sh = list(dr.shape)
                tl = wp.tile(sh, dr.dtype, tag="w" + k)
                nc.sync.dma_start(out=tl[:], in_=dr[:])
                wt[k] = tl
            for it in range(EPAD // PAIR):
                sl = slice(it * PAIR, (it + 1) * PAIR)
                ra = io.tile([128, PAIR], f16, tag="ra")
                rb = iob.tile([4, PAIR], f16, tag="rb")
                nc.sync.dma_start(out=ra[:], in_=inpa[:, sl])
                nc.sync.dma_start(out=rb[:], in_=inpb[:, sl])
                psd = ps2p.tile([128, PAIR], f32, tag="psd")
                for p in ("f", "t"):
                    ps0 = ps1p.tile([128, PAIR], f32, tag="ps")
                    for hh in range(2):
                        hs = slice(hh * 512, (hh + 1) * 512)
                        nc.tensor.matmul(out=ps0[:, hs], lhsT=wt[p + "w0a"][:],
                                         rhs=ra[:, hs], start=True, stop=False)
                        nc.tensor.matmul(out=ps0[:, hs], lhsT=wt[p + "w0b"][:],
                                         rhs=rb[:, hs], start=False, stop=True)
                    h1 = ap_.tile([128, PAIR], f16, tag="h1" + p)
                    nc.scalar.activation(h1[:], ps0[:], Tanh, bias=wt[p + "b0"][:, 0:1])
                    ps1 = ps1p.tile([128, PAIR], f32, tag="ps")
                    for hh in range(2):
                        hs = slice(hh * 512, (hh + 1) * 512)
                        nc.tensor.matmul(out=ps1[:, hs], lhsT=wt[p + "w1"][:],
                                         rhs=h1[:, hs], start=True, stop=True)
                    h2 = ap_.tile([128, PAIR], f16, tag="h2" + p)
                    nc.scalar.activation(h2[:], ps1[:], Tanh, bias=wt[p + "b1"][:, 0:1])
                    for hh in range(2):
                        hs = slice(hh * 512, (hh + 1) * 512)
                        if p == "f":
                            nc.tensor.matmul(out=psd[0:64, hs], lhsT=wt[p + "w2"][:],
                                             rhs=h2[:, hs], start=True, stop=True)
                        else:
                            nc.tensor.matmul(out=psd[64:128, hs], lhsT=wt[p + "w2"][:],
                                             rhs=h2[:, hs], start=True, stop=True,
                                             tile_position=(0, 64))
                dt_ = dtp.tile([128, PAIR], f16, tag="dt")
                nc.scalar.activation(dt_[:], psd[:], Tanh, bias=wt["bcat"][:, 0:1])
                nc.sync.dma_start(out=dout[:, sl], in_=dt_[:])

    # this walrus rejects any compute instruction carrying >1 sem wait;
    # hoist extra waits onto same-engine nops placed just before it.
    ctr = 0
    for bb in nc.main_func.blocks:
        new = []
        for ins in bb.instructions:
            si = getattr(ins, "sync_info", None)
            if si is not None and si.on_wait and len(si.on_wait) > 1:
                waits = list(si.on_wait)
                si.on_wait = [waits[-1]]
                for w in waits[:-1]:
                    ctr += 1
                    nop = mybir.InstNoOp(
                        name=f"wsplit-{ctr}", engine=ins.engine, ins=[], outs=[],
                        sync_info=mybir.SyncInfo(on_wait=[w], on_update=[]))
                    new.append(nop)
            new.append(ins)
        bb.instructions[:] = new
    return nc


def _get_cached_exec(nc):
    """Build (once) a jitted shard_map program across NCORES devices that does
    gather -> bass MLP kernel -> scatter-add -> psum -> tanh fully on device.

    Per-core device inputs: h16 [N, 64] fp16 (replicated), af/at [EPAD] int32,
    x16 [4, EPAD] fp16, weights.  Output: this core's [N/NCORES, 64] fp32 slice
    of tanh(delta_sum) -- the final answer, so the host does no gather/scatter
    and the wire carries no per-edge tensors.
    """
    if "exec" in _BASS_CACHE:
        return _BASS_CACHE["exec"]
    import jax
    import jax.numpy as jnp
    from jax.sharding import Mesh, PartitionSpec, NamedSharding
    from jax.experimental.shard_map import shard_map
    import concourse.mybir as mybir
    from concourse import bass2jax

    bass2jax.install_neuronx_cc_hook()
    assert nc.dbg_addr is None
    partition_name = (nc.partition_id_tensor.name
                      if nc.partition_id_tensor else None)

    in_names = []
    out_names = []
    out_avals = []
    for alloc in nc.m.functions[0].allocations:
        if not isinstance(alloc, mybir.MemoryLocationSet):
            continue
        name = alloc.memorylocations[0].name
        if alloc.kind == "ExternalInput":
            if name != partition_name:
                in_names.append(name)
        elif alloc.kind == "ExternalOutput":
            shape = tuple(alloc.tensor_shape)
            dtype = mybir.dt.np(alloc.dtype)
            out_avals.append(jax.core.ShapedArray(shape, dtype))
            out_names.append(name)
    all_names = in_names + out_names
    if partition_name is not None:
        all_names = all_names + [partition_name]
    NSL = N // NCORES

    devices = jax.devices()[:NCORES]
    mesh = Mesh(np.asarray(devices), ("core",))
    P = PartitionSpec

    # --- program A: all-gather h shards + gather + transpose to [128, EPAD] ---
    def _gather_body(h16s, af, at):
        h16 = jax.lax.all_gather(h16s, "core", axis=0, tiled=True)  # [N, 64]
        return jnp.concatenate([h16[af].T, h16[at].T], axis=0)

    fA = jax.jit(shard_map(_gather_body, mesh=mesh,
                           in_specs=(P("core"), P("core"), P("core")),
                           out_specs=P("core"), check_rep=False))

    # --- program B: the bare bass_exec custom call (hook requires the module
    # to contain nothing else; all operands must be jit parameters in order) ---
    def _bass_body(*args):
        operands = list(args)
        if partition_name is not None:
            operands.append(bass2jax.partition_id_tensor())
        outs = bass2jax._bass_exec_p.bind(
            *operands,
            out_avals=tuple(out_avals),
            in_names=tuple(all_names),
            out_names=tuple(out_names),
            lowering_input_output_aliases=(),
            sim_require_finite=True,
            sim_require_nnan=True,
            nc=nc,
        )
        return tuple(outs)

    spec_by_name = {"inpa": P("core"), "inpb": P("core")}
    b_in_specs = tuple(spec_by_name.get(n, P()) for n in in_names) + (P("core"),)
    fB = jax.jit(shard_map(_bass_body, mesh=mesh,
                           in_specs=b_in_specs,
                           out_specs=(P("core"),) * len(out_names),
                           check_rep=False))

    # --- program C: scatter-add + psum + final tanh, sliced per core ---
    def _scatter_body(d, af, at):
        dT = d.T                                       # [EPAD, 128]
        df = dT[:EPC, 0:64].astype(jnp.float32)
        dt = dT[:EPC, 64:128].astype(jnp.float32)
        delta = (jnp.zeros((N, D), jnp.float32)
                 .at[af[:EPC]].add(df)
                 .at[at[:EPC]].add(dt))
        delta = jax.lax.psum(delta, "core")
        c = jax.lax.axis_index("core")
        return jnp.tanh(jax.lax.dynamic_slice_in_dim(delta, c * NSL, NSL)
                        ).astype(jnp.float16)

    fC = jax.jit(shard_map(_scatter_body, mesh=mesh,
                           in_specs=(P("core"), P("core"), P("core")),
                           out_specs=P("core"), check_rep=False))

    # cached on-device zero ballast for the NEFF output binding (the kernel
    # writes every element of d, so the buffer contents are never read)
    zeros = jax.device_put(np.zeros((NCORES * 128, EPAD), np.float16),
                           NamedSharding(mesh, P("core")))
    shardings = {
        "idx": NamedSharding(mesh, P("core")),
        "row": NamedSharding(mesh, P("core", None)),
        "rep": NamedSharding(mesh, P()),
    }

    _BASS_CACHE["exec"] = (fA, fB, fC, in_names, out_names, zeros,
                           shardings, mesh)
    return _BASS_CACHE["exec"]


def _stage_inputs(addr_from, addr_to, h_local, h_global, x_local, x_global, t,
                  f_W0, f_b0, f_W1, f_b1, f_W2, f_b2,
                  t_W0, t_b0, t_W1, t_b1, t_W2, t_b2):
    """Host staging: per-core feature-major fp16 input maps + (af, at)."""
    from concurrent.futures import ThreadPoolExecutor

    af = np.asarray(addr_from).astype(np.int64)
    at = np.asarray(addr_to).astype(np.int64)
    h_local = np.asarray(h_local, np.float32)
    x_local = np.asarray(x_local, np.float32)
    const = np.concatenate([np.asarray(h_global, np.float32).ravel(),
                            np.asarray(x_global, np.float32).ravel(),
                            np.asarray(t, np.float32).ravel()])  # [21]

    weights = {}
    for p, W0, b0, W1, b1, W2, b2 in (
        ("f", f_W0, f_b0, f_W1, f_b1, f_W2, f_b2),
        ("t", t_W0, t_b0, t_W1, t_b1, t_W2, t_b2),
    ):
        W0 = np.asarray(W0, np.float32)
        b0eff = np.asarray(b0, np.float32) + const @ W0[132:153]
        weights[p + "w0a"] = np.ascontiguousarray(W0[0:128]).astype(np.float16)
        weights[p + "w0b"] = np.ascontiguousarray(W0[128:132]).astype(np.float16)
        weights[p + "w1"] = np.asarray(W1, np.float16)
        weights[p + "w2"] = np.asarray(W2, np.float16)
        weights[p + "b0"] = b0eff.reshape(H, 1).astype(np.float32)
        weights[p + "b1"] = np.asarray(b1, np.float32).reshape(H, 1)
    weights["bcat"] = np.concatenate([np.asarray(f_b2, np.float32).ravel(),
                                      np.asarray(t_b2, np.float32).ravel()]).reshape(128, 1)

    # feature-major fp16 staging: ia[0:64] = h[af].T, ia[64:128] = h[at].T
    h16T = np.ascontiguousarray(h_local.T).astype(np.float16)   # [64, N]
    x16T = np.ascontiguousarray(x_local.T).astype(np.float16)   # [4, E]

    ias = [None] * NCORES
    ibs = [None] * NCORES

    def _stage(c):
        s, e = c * EPC, (c + 1) * EPC
        ia = np.zeros((128, EPAD), np.float16)
        ia[0:64, :EPC] = h16T[:, af[s:e]]
        ia[64:128, :EPC] = h16T[:, at[s:e]]
        ib = np.zeros((4, EPAD), np.float16)
        ib[:, :EPC] = x16T[:, s:e]
        ias[c] = ia
        ibs[c] = ib

    with ThreadPoolExecutor(NCORES) as tp:
        list(tp.map(_stage, range(NCORES)))

    per_core = []
    for c in range(NCORES):
        m = {"inpa": ias[c], "inpb": ibs[c]}
        m.update(weights)
        per_core.append(m)
    return per_core, af, at


def _host_finish(dall, af, at):
    """dall: [NCORES, 128, EPAD] fp16 device outputs -> full [N, D] fp32."""
    from concurrent.futures import ThreadPoolExecutor
    deltaTs = [None] * NCORES

    def _post(c):
        s = c * EPC
        d32 = dall[c][:, :EPC].astype(np.float32)   # [128, EPC]
        dT = np.zeros((64, N), np.float32)
        for half, idx in ((0, af[s:s + EPC]), (1, at[s:s + EPC])):
            o = np.argsort(idx, kind="stable")
            si = idx[o]
            sv = d32[half * 64:(half + 1) * 64][:, o]
            starts = np.flatnonzero(np.r_[True, si[1:] != si[:-1]])
            sums = np.add.reduceat(sv, starts, axis=1)
            dT[:, si[starts]] += sums
        deltaTs[c] = dT

    with ThreadPoolExecutor(NCORES) as tp:
        list(tp.map(_post, range(NCORES)))

    total = deltaTs[0]
    for c in range(1, NCORES):
        total += deltaTs[c]
    return np.tanh(total.T).astype(np.float32)


def _make_weights(f_W0, f_b0, f_W1, f_b1, f_W2, f_b2,
                  t_W0, t_b0, t_W1, t_b1, t_W2, t_b2, const):
    weights = {}
    for p, W0, b0, W1, b1, W2, b2 in (
        ("f", f_W0, f_b0, f_W1, f_b1, f_W2, f_b2),
        ("t", t_W0, t_b0, t_W1, t_b1, t_W2, t_b2),
    ):
        W0 = np.asarray(W0, np.float32)
        b0eff = np.asarray(b0, np.float32) + const @ W0[132:153]
        weights[p + "w0a"] = np.ascontiguousarray(W0[0:128]).astype(np.float16)
        weights[p + "w0b"] = np.ascontiguousarray(W0[128:132]).astype(np.float16)
        weights[p + "w1"] = np.asarray(W1, np.float16)
        weights[p + "w2"] = np.asarray(W2, np.float16)
        weights[p + "b0"] = b0eff.reshape(H, 1).astype(np.float32)
        weights[p + "b1"] = np.asarray(b1, np.float32).reshape(H, 1)
    weights["bcat"] = np.concatenate([np.asarray(f_b2, np.float32).ravel(),
                                      np.asarray(t_b2, np.float32).ravel()]).reshape(128, 1)
    return weights


def _kernel_bass(addr_from, addr_to, h_local, h_global, x_local, x_global, t,
                 f_W0, f_b0, f_W1, f_b1, f_W2, f_b2,
                 t_W0, t_b0, t_W1, t_b1, t_W2, t_b2):
    import sys
    if "/opt/trn_rl_repo" not in sys.path:
        sys.path.insert(0, "/opt/trn_rl_repo")

    import jax

    if "nc" not in _BASS_CACHE:
        _BASS_CACHE["nc"] = _build_bass()
    nc = _BASS_CACHE["nc"]
    fA, fB, fC, in_names, out_names, zeros, shardings, mesh = _get_cached_exec(nc)

    af = np.asarray(addr_from).astype(np.int32)
    at = np.asarray(addr_to).astype(np.int32)
    const = np.concatenate([np.asarray(h_global, np.float32).ravel(),
                            np.asarray(x_global, np.float32).ravel(),
                            np.asarray(t, np.float32).ravel()])
    weights = _make_weights(f_W0, f_b0, f_W1, f_b1, f_W2, f_b2,
                            t_W0, t_b0, t_W1, t_b1, t_W2, t_b2, const)

    h16 = np.asarray(h_local, np.float32).astype(np.float16)        # [N, 64]
    afp = np.zeros((NCORES, EPAD), np.int32)
    atp = np.zeros((NCORES, EPAD), np.int32)
    afp[:, :EPC] = af.reshape(NCORES, EPC)
    atp[:, :EPC] = at.reshape(NCORES, EPC)
    x16 = np.zeros((NCORES * 4, EPAD), np.float16)
    xT = np.asarray(x_local, np.float32).astype(np.float16).T       # [4, E]
    for c in range(NCORES):
        x16[c * 4:(c + 1) * 4, :EPC] = xT[:, c * EPC:(c + 1) * EPC]

    # async upload of everything up front so transfers overlap execution
    dev_h = jax.device_put(h16, shardings["row"])
    dev_af = jax.device_put(afp.reshape(-1), shardings["idx"])
    dev_at = jax.device_put(atp.reshape(-1), shardings["idx"])
    dev_x = jax.device_put(x16, shardings["row"])
    dev_w = {k: jax.device_put(v, shardings["rep"]) for k, v in weights.items()}

    inpa = fA(dev_h, dev_af, dev_at)                   # device-resident
    vals = {"inpa": inpa, "inpb": dev_x}
    vals.update(dev_w)
    outs = fB(*[vals[n] for n in in_names], zeros)
    d = outs[out_names.index("d")]                     # device-resident
    out = fC(d, dev_af, dev_at)                        # [N, 64] fp16 sharded

    # parallel per-shard fetch
    from concurrent.futures import ThreadPoolExecutor
    shards = sorted(out.addressable_shards, key=lambda s: s.index[0].start or 0)
    res = np.empty((N, D), np.float32)
    NSL = N // NCORES

    def _fetch(i):
        res[i * NSL:(i + 1) * NSL] = np.asarray(shards[i].data, np.float32)

    with ThreadPoolExecutor(NCORES) as tp:
        list(tp.map(_fetch, range(NCORES)))
    return res


def kernel(**inputs):
    try:
        return _kernel_bass(**inputs)
    except Exception:
        import traceback
        traceback.print_exc()
        return _kernel_numpy(**inputs)
